# revision 10
# baseline (speedup 1.0000x reference)
"""Trainium2 Bass/Tile kernel for nn_MirrorAggregator.

Math (per batch, N=256 nodes, D=128 dim):
  alpha[n] = scale * s[n,:] @ (Wq1^T Wk1) @ m[n,:]^T
  sat_out  = s + alpha * (m - s)
  beta     = scale * (m @ (Wq2^T Wk2)) @ sat_out^T   (masked softmax over j)
  mir_out  = softmax(beta) @ m

Host folds each weight pair into one DxD constant (scale included):
  At = scale * Wk1^T @ Wq1    (v = m @ At, alpha = rowsum(v * s))
  Hs = scale * Wq2^T @ Wk2    (wT = Hs^T @ mT)

Design (v2, ~3.5x faster than the fp32 version):
 - Pure data parallel: 64 batches per core on 8 cores.
 - fp16 data path end-to-end (inputs cast host-side): PE runs all matmuls
   at 1 cycle/row (fp32 was 4), DVE gets 2x/4x modes, DMA bytes halve.
   Only the exp output (pt) is bf16 - e^beta reaches ~1e13 which overflows
   fp16's range; bf16 keeps fp32's exponent range.
 - Mask folded into a host-prepared m_masked tensor (masked rows zeroed,
   ones column = mask) that serves as the mir-GEMM rhs: masked satellites
   contribute nothing to numerator or denominator, so exp needs no bias
   and the softmax denominator rides the GEMM as a free 129th column.
 - alpha is stored to HBM (f32) and the host computes
   sat_out = s + alpha*(m-s) from the original f32 inputs; the device only
   needs sat for the attention (satT), never stores it. mir is stored
   unnormalized ([*, 129] = numerator | denominator) and the host divides.
 - Engine balance per batch (approx, from the TRN2 cost model):
   PE 855ns (transposes + 5 GEMMs), DVE ~930ns (evacs, diff, sat-stt,
   half the mir evac), ACT ~920ns (exp + wT evac), Pool ~930ns (gate
   dot-products, other half of mir evac), DMA ~850ns (16.2 MiB/core).
"""

import math
import os
import sys

import numpy as np

for _p in ("/opt/trn_rl_repo",):
    if os.path.isdir(_p) and _p not in sys.path:
        sys.path.insert(0, _p)

import ml_dtypes

import concourse.bacc as bacc
import concourse.tile as tile
from concourse import mybir
from concourse.bass_utils import run_bass_kernel_spmd
from concourse.masks import make_identity

B, N, D = 512, 256, 128
NCORES = 8
BL = B // NCORES          # batches per core
NBLK = BL * 2             # 128-row blocks per core
CH = 16                   # batches per DMA chunk (8KB per-partition lines)
F32 = mybir.dt.float32
F16 = mybir.dt.float16
BF16 = mybir.dt.bfloat16

_CACHE = {}


def _build(bl=BL):
    assert bl % CH == 0
    nblk = bl * 2
    nc = bacc.Bacc(None, target_bir_lowering=False)
    mr_d = nc.declare_dram_parameter("mr", [128, nblk, 128], F16, isOutput=False)
    mm_d = nc.declare_dram_parameter("mm", [128, nblk, 129], BF16, isOutput=False)
    sr_d = nc.declare_dram_parameter("sr", [128, nblk, 128], F16, isOutput=False)
    at_d = nc.declare_dram_parameter("At", [128, 128], F16, isOutput=False)
    hs_d = nc.declare_dram_parameter("Hs", [128, 128], F16, isOutput=False)
    al_d = nc.declare_dram_parameter("alpha", [128, 2 * bl], F32, isOutput=True)
    mir_d = nc.declare_dram_parameter("mir_out", [128, nblk, 129], BF16, isOutput=True)

    mult = mybir.AluOpType.mult
    add = mybir.AluOpType.add
    sub = mybir.AluOpType.subtract
    Exp = mybir.ActivationFunctionType.Exp

    with tile.TileContext(nc) as tc:
        with (
            tc.tile_pool(name="const", bufs=1) as const,
            tc.tile_pool(name="io", bufs=3) as io,
            tc.tile_pool(name="work", bufs=2) as work,
            tc.tile_pool(name="ps_tp", bufs=2, space="PSUM") as ps_tp,
            tc.tile_pool(name="ps_w", bufs=1, space="PSUM") as ps_w,
            tc.tile_pool(name="ps_v", bufs=1, space="PSUM") as ps_v,
            tc.tile_pool(name="ps_b", bufs=2, space="PSUM") as ps_b,
            tc.tile_pool(name="ps_m", bufs=1, space="PSUM") as ps_m,
        ):
            ident = const.tile([128, 128], F16)
            make_identity(nc, ident)
            at_r = const.tile([128, 128], F16)
            nc.sync.dma_start(out=at_r[:], in_=at_d[:])
            hs_r = const.tile([128, 128], F16)
            nc.sync.dma_start(out=hs_r[:], in_=hs_d[:])
            alpha_all = const.tile([128, 2 * bl], F32)

            for it in range(bl // CH):
                blk0 = it * 2 * CH
                m_p = io.tile([128, 2 * CH, 128], F16, tag="m_p")
                nc.sync.dma_start(out=m_p[:], in_=mr_d[:, blk0:blk0 + 2 * CH, :])
                mm_p = io.tile([128, 2 * CH, 129], BF16, tag="mm_p")
                nc.sync.dma_start(out=mm_p[:], in_=mm_d[:, blk0:blk0 + 2 * CH, :])
                s_p = io.tile([128, 2 * CH, 128], F16, tag="s_p")
                nc.sync.dma_start(out=s_p[:], in_=sr_d[:, blk0:blk0 + 2 * CH, :])
                mir_s = io.tile([128, 2 * CH, 129], BF16, tag="mir_s")

                for pb in range(CH // 2):
                    base = pb * 4          # block offset within chunk
                    # ---- mT via PE transpose, evacuate once per pair ----
                    tpm = ps_tp.tile([128, 512], F16, tag="tp", name="tpm")
                    for k in range(4):
                        nc.tensor.transpose(
                            tpm[:, k * 128:(k + 1) * 128],
                            m_p[:, base + k, :], ident[:])
                    mTs = work.tile([128, 512], F16, tag="mTs")
                    nc.vector.tensor_copy(out=mTs[:], in_=tpm[:])

                    # ---- wT = Hs^T @ mT for both batches in one GEMM ----
                    wp = ps_w.tile([128, 512], F32, tag="wp")
                    nc.tensor.matmul(wp[:], hs_r[:], mTs[:], start=True, stop=True)
                    wTs = work.tile([128, 512], F16, tag="wTs")
                    nc.scalar.copy(out=wTs[:], in_=wp[:])

                    # ---- v = m @ At (row layout), 4 x 128-wide ----
                    vp = ps_v.tile([128, 4, 128], F32, tag="vp")
                    for k in range(4):
                        nc.tensor.matmul(
                            vp[:, k, :],
                            mTs[:, k * 128:(k + 1) * 128],
                            at_r[:], start=True, stop=True)

                    # ---- diff = m - s (Pool: its only PSUM-free job) ----
                    diff = work.tile([128, 4, 128], F16, tag="diff")
                    nc.gpsimd.tensor_tensor(
                        out=diff[:], in0=m_p[:, base:base + 4, :],
                        in1=s_p[:, base:base + 4, :], op=sub)

                    # ---- gate: prod = v*s (one DVE op), then per-block
                    # innermost-dim reductions into alpha columns ----
                    prod = work.tile([128, 4, 128], F16, tag="prod")
                    nc.vector.tensor_tensor(
                        out=prod[:], in0=vp[:],
                        in1=s_p[:, base:base + 4, :], op=mult)
                    for k in range(4):
                        b = it * CH + pb * 2 + (k // 2)
                        col = b * 2 + (k % 2)
                        nc.vector.tensor_reduce(
                            out=alpha_all[:, col:col + 1], in_=prod[:, k, :],
                            axis=mybir.AxisListType.X, op=add)
                    # ---- sat = s + alpha*diff: 4 scalar-mults + one add ----
                    satm = work.tile([128, 4, 128], F16, tag="satm")
                    for k in range(4):
                        b = it * CH + pb * 2 + (k // 2)
                        col = b * 2 + (k % 2)
                        nc.vector.tensor_scalar(
                            out=satm[:, k, :], in0=diff[:, k, :],
                            scalar1=alpha_all[:, col:col + 1], scalar2=None,
                            op0=mult)
                    sat_p = work.tile([128, 4, 128], F16, tag="sat_p")
                    nc.vector.tensor_tensor(
                        out=sat_p[:], in0=satm[:],
                        in1=s_p[:, base:base + 4, :], op=add)

                    # ---- satT via PE transpose, evacuate once per pair ----
                    tps = ps_tp.tile([128, 512], F16, tag="tp", name="tps")
                    for k in range(4):
                        nc.tensor.transpose(
                            tps[:, k * 128:(k + 1) * 128], sat_p[:, k, :],
                            ident[:])
                    satTs = work.tile([128, 512], F16, tag="satTs")
                    nc.vector.tensor_copy(out=satTs[:], in_=tps[:])

                    # ---- betaT[j,i] then exp, per batch (2-deep PSUM) ----
                    pts = []
                    for q in range(2):
                        bp = ps_b.tile([128, 2, 256], F32, tag="bp", name="bp")
                        for jc in range(2):
                            nc.tensor.matmul(
                                bp[:, jc, :],
                                satTs[:, (q * 2 + jc) * 128:(q * 2 + jc + 1) * 128],
                                wTs[:, q * 256:(q + 1) * 256],
                                start=True, stop=True)
                        pt = work.tile([128, 2, 256], BF16, tag=f"pt{q}",
                                       name=f"pt{q}")
                        nc.scalar.activation(
                            out=pt[:], in_=bp[:], func=Exp, bias=0.0, scale=1.0)
                        pts.append(pt)

                    # ---- mir numerator | denominator via [m|mask] ----
                    # mp rows padded to 256 f32 so each accumulation group
                    # stays inside one 2KB PSUM bank (516B groups at 1548B
                    # offsets silently corrupt across the bank boundary).
                    mp = ps_m.tile([128, 4, 256], F32, tag="mp", name="mp")
                    for q in range(2):
                        for h in range(2):
                            for jc in range(2):
                                nc.tensor.matmul(
                                    mp[:, q * 2 + h, 0:129],
                                    pts[q][:, jc, h * 128:(h + 1) * 128],
                                    mm_p[:, base + q * 2 + jc, :],
                                    start=(jc == 0), stop=(jc == 1))
                    nc.scalar.copy(
                        out=mir_s[:, base:base + 4, :], in_=mp[:, :, 0:129])

                nc.sync.dma_start(
                    out=mir_d[:, blk0:blk0 + 2 * CH, :], in_=mir_s[:])

            nc.sync.dma_start(out=al_d[:], in_=alpha_all[:])
    nc.finalize()
    return nc


def _get_nc():
    if "nc" not in _CACHE:
        _CACHE["nc"] = _build()
    return _CACHE["nc"]


def _to_pblk(a, x):
    """[BL, N, x] -> [128, NBLK, x] partition-major block layout."""
    return np.ascontiguousarray(
        a.reshape(BL, 2, 128, x).transpose(2, 0, 1, 3).reshape(128, NBLK, x))


def _from_pblk(a, x):
    """[128, NBLK, x] -> [BL, N, x]."""
    return a.reshape(128, BL, 2, x).transpose(1, 2, 0, 3).reshape(BL, N, x)


def run(inputs, trace=False, **kw):
    mirror = np.asarray(inputs["mirror_nodes"], dtype=np.float32)
    sat = np.asarray(inputs["satellite_nodes"], dtype=np.float32)
    mask = np.asarray(inputs["satellite_node_mask"])
    Wq1 = np.asarray(inputs["Wq1"], dtype=np.float64)
    Wk1 = np.asarray(inputs["Wk1"], dtype=np.float64)
    Wq2 = np.asarray(inputs["Wq2"], dtype=np.float64)
    Wk2 = np.asarray(inputs["Wk2"], dtype=np.float64)

    scale = 1.0 / math.sqrt(D)
    At = (scale * (Wk1.T @ Wq1)).astype(np.float16)
    Hs = (scale * (Wq2.T @ Wk2)).astype(np.float16)

    m16 = mirror.astype(np.float16)
    s16 = sat.astype(np.float16)
    mbf = mirror.astype(ml_dtypes.bfloat16)
    mm16 = np.concatenate(
        [np.where(mask[..., None], mbf, ml_dtypes.bfloat16(0.0)),
         mask[..., None].astype(ml_dtypes.bfloat16)], axis=2)

    nc = _get_nc()
    in_maps = []
    for c in range(NCORES):
        lo, hi = c * BL, (c + 1) * BL
        in_maps.append({
            "mr": _to_pblk(m16[lo:hi], 128),
            "mm": _to_pblk(mm16[lo:hi], 129),
            "sr": _to_pblk(s16[lo:hi], 128),
            "At": np.ascontiguousarray(At),
            "Hs": np.ascontiguousarray(Hs),
        })
    res = run_bass_kernel_spmd(nc, in_maps, list(range(NCORES)), trace=trace, **kw)

    sat_parts, mir_parts = [], []
    for c, r in enumerate(res.results):
        lo, hi = c * BL, (c + 1) * BL
        # alpha [128, 2*BL] -> [BL, N]
        al = np.asarray(r["alpha"], dtype=np.float32)
        al = al.reshape(128, BL, 2).transpose(1, 2, 0).reshape(BL, N)
        sat_parts.append(sat[lo:hi] + al[..., None] * (mirror[lo:hi] - sat[lo:hi]))
        mir_u = _from_pblk(
            np.asarray(r["mir_out"]).astype(np.float32), 129)
        mir_parts.append(mir_u[..., :128] / mir_u[..., 128:129])
    sat_out = np.concatenate(sat_parts, axis=0)
    mir_out = np.concatenate(mir_parts, axis=0)
    return (sat_out, mir_out), res


def kernel(**inputs):
    out, _ = run(inputs)
    return out


# revision 12
# speedup vs baseline: 1.2187x; 1.2187x over previous
"""Trainium2 Bass/Tile kernel for nn_MirrorAggregator.

Math (per batch, N=256 nodes, D=128 dim):
  alpha[n] = scale * s[n,:] @ (Wq1^T Wk1) @ m[n,:]^T
  sat_out  = s + alpha * (m - s)
  beta     = scale * (m @ (Wq2^T Wk2)) @ sat_out^T   (masked softmax over j)
  mir_out  = softmax(beta) @ m

Host folds each weight pair into one DxD constant (scale included):
  At = scale * Wk1^T @ Wq1    (v = m @ At, alpha = rowsum(v * s))
  Hs = scale * Wq2^T @ Wk2    (wT = Hs^T @ mT)

Design (v2, ~3.5x faster than the fp32 version):
 - Pure data parallel: 64 batches per core on 8 cores.
 - fp16 data path end-to-end (inputs cast host-side): PE runs all matmuls
   at 1 cycle/row (fp32 was 4), DVE gets 2x/4x modes, DMA bytes halve.
   Only the exp output (pt) is bf16 - e^beta reaches ~1e13 which overflows
   fp16's range; bf16 keeps fp32's exponent range.
 - Mask folded into a host-prepared m_masked tensor (masked rows zeroed,
   ones column = mask) that serves as the mir-GEMM rhs: masked satellites
   contribute nothing to numerator or denominator, so exp needs no bias
   and the softmax denominator rides the GEMM as a free 129th column.
 - alpha is stored to HBM (f32) and the host computes
   sat_out = s + alpha*(m-s) from the original f32 inputs; the device only
   needs sat for the attention (satT), never stores it. mir is stored
   unnormalized ([*, 129] = numerator | denominator) and the host divides.
 - Engine balance per batch (approx, from the TRN2 cost model):
   PE 855ns (transposes + 5 GEMMs), DVE ~930ns (evacs, diff, sat-stt,
   half the mir evac), ACT ~920ns (exp + wT evac), Pool ~930ns (gate
   dot-products, other half of mir evac), DMA ~850ns (16.2 MiB/core).
"""

import math
import os
import sys

import numpy as np

for _p in ("/opt/trn_rl_repo",):
    if os.path.isdir(_p) and _p not in sys.path:
        sys.path.insert(0, _p)

import ml_dtypes

import concourse.bacc as bacc
import concourse.tile as tile
from concourse import mybir
from concourse.bass_utils import run_bass_kernel_spmd
from concourse.masks import make_identity

B, N, D = 512, 256, 128
NCORES = 8
BL = B // NCORES          # batches per core
NBLK = BL * 2             # 128-row blocks per core
CH = 16                   # batches per DMA chunk (8KB per-partition lines)
F32 = mybir.dt.float32
F16 = mybir.dt.float16
BF16 = mybir.dt.bfloat16

_CACHE = {}


def _build(bl=BL):
    assert bl % CH == 0
    nblk = bl * 2
    nc = bacc.Bacc(None, target_bir_lowering=False)
    mr_d = nc.declare_dram_parameter("mr", [128, nblk, 128], F16, isOutput=False)
    mm_d = nc.declare_dram_parameter("mm", [128, nblk, 129], BF16, isOutput=False)
    sr_d = nc.declare_dram_parameter("sr", [128, nblk, 128], F16, isOutput=False)
    at_d = nc.declare_dram_parameter("At", [128, 128], F16, isOutput=False)
    hs_d = nc.declare_dram_parameter("Hs", [128, 128], F16, isOutput=False)
    al_d = nc.declare_dram_parameter("alpha", [128, 2 * bl], F32, isOutput=True)
    mir_d = nc.declare_dram_parameter("mir_out", [128, nblk, 129], BF16, isOutput=True)

    mult = mybir.AluOpType.mult
    add = mybir.AluOpType.add
    sub = mybir.AluOpType.subtract
    Exp = mybir.ActivationFunctionType.Exp

    with tile.TileContext(nc) as tc:
        with (
            tc.tile_pool(name="const", bufs=1) as const,
            tc.tile_pool(name="io", bufs=3) as io,
            tc.tile_pool(name="work", bufs=2) as work,
            tc.tile_pool(name="ps_tp", bufs=2, space="PSUM") as ps_tp,
            tc.tile_pool(name="ps_w", bufs=1, space="PSUM") as ps_w,
            tc.tile_pool(name="ps_v", bufs=1, space="PSUM") as ps_v,
            tc.tile_pool(name="ps_b", bufs=2, space="PSUM") as ps_b,
            tc.tile_pool(name="ps_m", bufs=1, space="PSUM") as ps_m,
        ):
            ident = const.tile([128, 128], F16)
            make_identity(nc, ident)
            at_r = const.tile([128, 128], F16)
            nc.sync.dma_start(out=at_r[:], in_=at_d[:])
            hs_r = const.tile([128, 128], F16)
            nc.sync.dma_start(out=hs_r[:], in_=hs_d[:])
            alpha_all = const.tile([128, 2 * bl], F32)

            for it in range(bl // CH):
                blk0 = it * 2 * CH
                m_p = io.tile([128, 2 * CH, 128], F16, tag="m_p")
                nc.sync.dma_start(out=m_p[:], in_=mr_d[:, blk0:blk0 + 2 * CH, :])
                mm_p = io.tile([128, 2 * CH, 129], BF16, tag="mm_p")
                nc.sync.dma_start(out=mm_p[:], in_=mm_d[:, blk0:blk0 + 2 * CH, :])
                s_p = io.tile([128, 2 * CH, 128], F16, tag="s_p")
                nc.sync.dma_start(out=s_p[:], in_=sr_d[:, blk0:blk0 + 2 * CH, :])
                mir_s = io.tile([128, 2 * CH, 129], BF16, tag="mir_s")

                for pb in range(CH // 2):
                    base = pb * 4          # block offset within chunk
                    # ---- mT via PE transpose, evacuate once per pair ----
                    tpm = ps_tp.tile([128, 512], F16, tag="tp", name="tpm")
                    for k in range(4):
                        nc.tensor.transpose(
                            tpm[:, k * 128:(k + 1) * 128],
                            m_p[:, base + k, :], ident[:])
                    mTs = work.tile([128, 512], F16, tag="mTs")
                    nc.vector.tensor_copy(out=mTs[:], in_=tpm[:])

                    # ---- wT = Hs^T @ mT for both batches in one GEMM ----
                    wp = ps_w.tile([128, 512], F32, tag="wp")
                    nc.tensor.matmul(wp[:], hs_r[:], mTs[:], start=True, stop=True)
                    wTs = work.tile([128, 512], F16, tag="wTs")
                    nc.scalar.copy(out=wTs[:], in_=wp[:])

                    # ---- v = m @ At (row layout), 4 x 128-wide ----
                    vp = ps_v.tile([128, 4, 128], F32, tag="vp")
                    for k in range(4):
                        nc.tensor.matmul(
                            vp[:, k, :],
                            mTs[:, k * 128:(k + 1) * 128],
                            at_r[:], start=True, stop=True)

                    # ---- diff = m - s (Pool: its only PSUM-free job) ----
                    diff = work.tile([128, 4, 128], F16, tag="diff")
                    nc.gpsimd.tensor_tensor(
                        out=diff[:], in0=m_p[:, base:base + 4, :],
                        in1=s_p[:, base:base + 4, :], op=sub)

                    # ---- gate: prod = v*s (one DVE op), then per-block
                    # innermost-dim reductions into alpha columns ----
                    prod = work.tile([128, 4, 128], F16, tag="prod")
                    nc.vector.tensor_tensor(
                        out=prod[:], in0=vp[:],
                        in1=s_p[:, base:base + 4, :], op=mult)
                    col0 = (it * CH + pb * 2) * 2
                    nc.vector.tensor_reduce(
                        out=alpha_all[:, col0:col0 + 4], in_=prod[:],
                        axis=mybir.AxisListType.X, op=add)
                    # ---- sat = s + alpha*diff (fused stt per block) ----
                    sat_p = work.tile([128, 4, 128], F16, tag="sat_p")
                    for k in range(4):
                        nc.vector.scalar_tensor_tensor(
                            out=sat_p[:, k, :], in0=diff[:, k, :],
                            scalar=alpha_all[:, col0 + k:col0 + k + 1],
                            in1=s_p[:, base + k, :], op0=mult, op1=add)

                    # ---- satT via PE transpose, evacuate once per pair ----
                    tps = ps_tp.tile([128, 512], F16, tag="tp", name="tps")
                    for k in range(4):
                        nc.tensor.transpose(
                            tps[:, k * 128:(k + 1) * 128], sat_p[:, k, :],
                            ident[:])
                    satTs = work.tile([128, 512], F16, tag="satTs")
                    nc.vector.tensor_copy(out=satTs[:], in_=tps[:])

                    # ---- betaT[j,i] then exp, per batch (2-deep PSUM) ----
                    pts = []
                    for q in range(2):
                        bp = ps_b.tile([128, 2, 256], F32, tag="bp", name="bp")
                        for jc in range(2):
                            nc.tensor.matmul(
                                bp[:, jc, :],
                                satTs[:, (q * 2 + jc) * 128:(q * 2 + jc + 1) * 128],
                                wTs[:, q * 256:(q + 1) * 256],
                                start=True, stop=True)
                        pt = work.tile([128, 2, 256], BF16, tag=f"pt{q}",
                                       name=f"pt{q}")
                        nc.scalar.activation(
                            out=pt[:], in_=bp[:], func=Exp, bias=0.0, scale=1.0)
                        pts.append(pt)

                    # ---- mir numerator | denominator via [m|mask] ----
                    # mp rows padded to 256 f32 so each accumulation group
                    # stays inside one 2KB PSUM bank (516B groups at 1548B
                    # offsets silently corrupt across the bank boundary).
                    mp = ps_m.tile([128, 4, 256], F32, tag="mp", name="mp")
                    for q in range(2):
                        for h in range(2):
                            for jc in range(2):
                                nc.tensor.matmul(
                                    mp[:, q * 2 + h, 0:129],
                                    pts[q][:, jc, h * 128:(h + 1) * 128],
                                    mm_p[:, base + q * 2 + jc, :],
                                    start=(jc == 0), stop=(jc == 1))
                    nc.scalar.copy(
                        out=mir_s[:, base:base + 4, :], in_=mp[:, :, 0:129])

                nc.sync.dma_start(
                    out=mir_d[:, blk0:blk0 + 2 * CH, :], in_=mir_s[:])

            nc.sync.dma_start(out=al_d[:], in_=alpha_all[:])
    nc.finalize()
    return nc


def _get_nc():
    if "nc" not in _CACHE:
        _CACHE["nc"] = _build()
    return _CACHE["nc"]


def _to_pblk(a, x):
    """[BL, N, x] -> [128, NBLK, x] partition-major block layout."""
    return np.ascontiguousarray(
        a.reshape(BL, 2, 128, x).transpose(2, 0, 1, 3).reshape(128, NBLK, x))


def _from_pblk(a, x):
    """[128, NBLK, x] -> [BL, N, x]."""
    return a.reshape(128, BL, 2, x).transpose(1, 2, 0, 3).reshape(BL, N, x)


def run(inputs, trace=False, **kw):
    mirror = np.asarray(inputs["mirror_nodes"], dtype=np.float32)
    sat = np.asarray(inputs["satellite_nodes"], dtype=np.float32)
    mask = np.asarray(inputs["satellite_node_mask"])
    Wq1 = np.asarray(inputs["Wq1"], dtype=np.float64)
    Wk1 = np.asarray(inputs["Wk1"], dtype=np.float64)
    Wq2 = np.asarray(inputs["Wq2"], dtype=np.float64)
    Wk2 = np.asarray(inputs["Wk2"], dtype=np.float64)

    scale = 1.0 / math.sqrt(D)
    At = (scale * (Wk1.T @ Wq1)).astype(np.float16)
    Hs = (scale * (Wq2.T @ Wk2)).astype(np.float16)

    m16 = mirror.astype(np.float16)
    s16 = sat.astype(np.float16)
    mbf = mirror.astype(ml_dtypes.bfloat16)
    mm16 = np.concatenate(
        [np.where(mask[..., None], mbf, ml_dtypes.bfloat16(0.0)),
         mask[..., None].astype(ml_dtypes.bfloat16)], axis=2)

    nc = _get_nc()
    in_maps = []
    for c in range(NCORES):
        lo, hi = c * BL, (c + 1) * BL
        in_maps.append({
            "mr": _to_pblk(m16[lo:hi], 128),
            "mm": _to_pblk(mm16[lo:hi], 129),
            "sr": _to_pblk(s16[lo:hi], 128),
            "At": np.ascontiguousarray(At),
            "Hs": np.ascontiguousarray(Hs),
        })
    res = run_bass_kernel_spmd(nc, in_maps, list(range(NCORES)), trace=trace, **kw)

    sat_parts, mir_parts = [], []
    for c, r in enumerate(res.results):
        lo, hi = c * BL, (c + 1) * BL
        # alpha [128, 2*BL] -> [BL, N]
        al = np.asarray(r["alpha"], dtype=np.float32)
        al = al.reshape(128, BL, 2).transpose(1, 2, 0).reshape(BL, N)
        sat_parts.append(sat[lo:hi] + al[..., None] * (mirror[lo:hi] - sat[lo:hi]))
        mir_u = _from_pblk(
            np.asarray(r["mir_out"]).astype(np.float32), 129)
        mir_parts.append(mir_u[..., :128] / mir_u[..., 128:129])
    sat_out = np.concatenate(sat_parts, axis=0)
    mir_out = np.concatenate(mir_parts, axis=0)
    return (sat_out, mir_out), res


def kernel(**inputs):
    out, _ = run(inputs)
    return out


# revision 13
# speedup vs baseline: 1.3042x; 1.0702x over previous
"""Trainium2 Bass/Tile kernel for nn_MirrorAggregator.

Math (per batch, N=256 nodes, D=128 dim):
  alpha[n] = scale * s[n,:] @ (Wq1^T Wk1) @ m[n,:]^T
  sat_out  = s + alpha * (m - s)
  beta     = scale * (m @ (Wq2^T Wk2)) @ sat_out^T   (masked softmax over j)
  mir_out  = softmax(beta) @ m

Host folds each weight pair into one DxD constant (scale included):
  At = scale * Wk1^T @ Wq1    (v = m @ At, alpha = rowsum(v * s))
  Hs = scale * Wq2^T @ Wk2    (wT = Hs^T @ mT)

Design (v2, ~3.5x faster than the fp32 version):
 - Pure data parallel: 64 batches per core on 8 cores.
 - fp16 data path end-to-end (inputs cast host-side): PE runs all matmuls
   at 1 cycle/row (fp32 was 4), DVE gets 2x/4x modes, DMA bytes halve.
   Only the exp output (pt) is bf16 - e^beta reaches ~1e13 which overflows
   fp16's range; bf16 keeps fp32's exponent range.
 - Mask folded into a host-prepared m_masked tensor (masked rows zeroed,
   ones column = mask) that serves as the mir-GEMM rhs: masked satellites
   contribute nothing to numerator or denominator, so exp needs no bias
   and the softmax denominator rides the GEMM as a free 129th column.
 - alpha is stored to HBM (f32) and the host computes
   sat_out = s + alpha*(m-s) from the original f32 inputs; the device only
   needs sat for the attention (satT), never stores it. mir is stored
   unnormalized ([*, 129] = numerator | denominator) and the host divides.
 - Engine balance per batch (approx, from the TRN2 cost model):
   PE 855ns (transposes + 5 GEMMs), DVE ~930ns (evacs, diff, sat-stt,
   half the mir evac), ACT ~920ns (exp + wT evac), Pool ~930ns (gate
   dot-products, other half of mir evac), DMA ~850ns (16.2 MiB/core).
"""

import math
import os
import sys

import numpy as np

for _p in ("/opt/trn_rl_repo",):
    if os.path.isdir(_p) and _p not in sys.path:
        sys.path.insert(0, _p)

import ml_dtypes

import concourse.bacc as bacc
import concourse.tile as tile
from concourse import mybir
from concourse.bass_utils import run_bass_kernel_spmd
from concourse.masks import make_identity

B, N, D = 512, 256, 128
NCORES = 8
BL = B // NCORES          # batches per core
NBLK = BL * 2             # 128-row blocks per core
CH = 16                   # batches per DMA chunk (8KB per-partition lines)
F32 = mybir.dt.float32
F16 = mybir.dt.float16
BF16 = mybir.dt.bfloat16

_CACHE = {}


def _build(bl=BL):
    assert bl % CH == 0
    nblk = bl * 2
    nc = bacc.Bacc(None, target_bir_lowering=False)
    mr_d = nc.declare_dram_parameter("mr", [128, nblk, 128], F16, isOutput=False)
    mm_d = nc.declare_dram_parameter("mm", [128, nblk, 129], BF16, isOutput=False)
    sr_d = nc.declare_dram_parameter("sr", [128, nblk, 128], F16, isOutput=False)
    at_d = nc.declare_dram_parameter("At", [128, 128], F16, isOutput=False)
    hs_d = nc.declare_dram_parameter("Hs", [128, 128], F16, isOutput=False)
    al_d = nc.declare_dram_parameter("alpha", [128, 2 * bl], F32, isOutput=True)
    mir_d = nc.declare_dram_parameter("mir_out", [128, nblk, 129], BF16, isOutput=True)

    mult = mybir.AluOpType.mult
    add = mybir.AluOpType.add
    sub = mybir.AluOpType.subtract
    Exp = mybir.ActivationFunctionType.Exp

    with tile.TileContext(nc) as tc:
        with (
            tc.tile_pool(name="const", bufs=1) as const,
            tc.tile_pool(name="io", bufs=3) as io,
            tc.tile_pool(name="work", bufs=2) as work,
            tc.tile_pool(name="ps_tp", bufs=2, space="PSUM") as ps_tp,
            tc.tile_pool(name="ps_w", bufs=1, space="PSUM") as ps_w,
            tc.tile_pool(name="ps_v", bufs=1, space="PSUM") as ps_v,
            tc.tile_pool(name="ps_b", bufs=2, space="PSUM") as ps_b,
            tc.tile_pool(name="ps_m", bufs=1, space="PSUM") as ps_m,
        ):
            ident = const.tile([128, 128], F16)
            make_identity(nc, ident)
            at_r = const.tile([128, 128], F16)
            nc.sync.dma_start(out=at_r[:], in_=at_d[:])
            hs_r = const.tile([128, 128], F16)
            nc.sync.dma_start(out=hs_r[:], in_=hs_d[:])
            alpha_all = const.tile([128, 2 * bl], F32)

            for it in range(bl // CH):
                blk0 = it * 2 * CH
                m_p = io.tile([128, 2 * CH, 128], F16, tag="m_p")
                nc.sync.dma_start(out=m_p[:], in_=mr_d[:, blk0:blk0 + 2 * CH, :])
                mm_p = io.tile([128, 2 * CH, 129], BF16, tag="mm_p")
                nc.sync.dma_start(out=mm_p[:], in_=mm_d[:, blk0:blk0 + 2 * CH, :])
                s_p = io.tile([128, 2 * CH, 128], F16, tag="s_p")
                nc.sync.dma_start(out=s_p[:], in_=sr_d[:, blk0:blk0 + 2 * CH, :])
                mir_s = io.tile([128, 2 * CH, 129], BF16, tag="mir_s")

                # ===== pass 1: gate + sat + transposed operands for the
                # whole chunk.  Uses only tp/wp/vp PSUM pools, so the tile
                # scheduler can overlap it with pass 2 of the previous
                # chunk (which uses bp/mp). =====
                wTs_c = work.tile([128, CH // 2, 512], F16, tag="wTs_c")
                satTs_c = work.tile([128, CH // 2, 512], F16, tag="satTs_c")
                for pb in range(CH // 2):
                    base = pb * 4          # block offset within chunk
                    # ---- mT via PE transpose, evacuate once per pair ----
                    tpm = ps_tp.tile([128, 512], F16, tag="tp", name="tpm")
                    for k in range(4):
                        nc.tensor.transpose(
                            tpm[:, k * 128:(k + 1) * 128],
                            m_p[:, base + k, :], ident[:])
                    mTs = work.tile([128, 512], F16, tag="mTs")
                    nc.vector.tensor_copy(out=mTs[:], in_=tpm[:])

                    # ---- wT = Hs^T @ mT for both batches in one GEMM ----
                    wp = ps_w.tile([128, 512], F32, tag="wp")
                    nc.tensor.matmul(wp[:], hs_r[:], mTs[:], start=True, stop=True)
                    nc.scalar.copy(out=wTs_c[:, pb, :], in_=wp[:])

                    # ---- v = m @ At (row layout), 4 x 128-wide ----
                    vp = ps_v.tile([128, 4, 128], F32, tag="vp")
                    for k in range(4):
                        nc.tensor.matmul(
                            vp[:, k, :],
                            mTs[:, k * 128:(k + 1) * 128],
                            at_r[:], start=True, stop=True)

                    # ---- diff = m - s (Pool: its only PSUM-free job) ----
                    diff = work.tile([128, 4, 128], F16, tag="diff")
                    nc.gpsimd.tensor_tensor(
                        out=diff[:], in0=m_p[:, base:base + 4, :],
                        in1=s_p[:, base:base + 4, :], op=sub)

                    # ---- gate: prod = v*s, one reduction for 4 columns ----
                    prod = work.tile([128, 4, 128], F16, tag="prod")
                    nc.vector.tensor_tensor(
                        out=prod[:], in0=vp[:],
                        in1=s_p[:, base:base + 4, :], op=mult)
                    col0 = (it * CH + pb * 2) * 2
                    nc.vector.tensor_reduce(
                        out=alpha_all[:, col0:col0 + 4], in_=prod[:],
                        axis=mybir.AxisListType.X, op=add)
                    # ---- sat = s + alpha*diff (fused stt per block) ----
                    sat_p = work.tile([128, 4, 128], F16, tag="sat_p")
                    for k in range(4):
                        nc.vector.scalar_tensor_tensor(
                            out=sat_p[:, k, :], in0=diff[:, k, :],
                            scalar=alpha_all[:, col0 + k:col0 + k + 1],
                            in1=s_p[:, base + k, :], op0=mult, op1=add)

                    # ---- satT via PE transpose, evacuate once per pair ----
                    tps = ps_tp.tile([128, 512], F16, tag="tp", name="tps")
                    for k in range(4):
                        nc.tensor.transpose(
                            tps[:, k * 128:(k + 1) * 128], sat_p[:, k, :],
                            ident[:])
                    nc.vector.tensor_copy(out=satTs_c[:, pb, :], in_=tps[:])

                # ===== pass 2: attention + aggregation for the chunk.
                # Uses only bp/mp PSUM pools. =====
                for pb in range(CH // 2):
                    base = pb * 4
                    pts = []
                    for q in range(2):
                        bp = ps_b.tile([128, 2, 256], F32, tag="bp", name="bp")
                        for jc in range(2):
                            nc.tensor.matmul(
                                bp[:, jc, :],
                                satTs_c[:, pb,
                                        (q * 2 + jc) * 128:(q * 2 + jc + 1) * 128],
                                wTs_c[:, pb, q * 256:(q + 1) * 256],
                                start=True, stop=True)
                        pt = work.tile([128, 2, 256], BF16, tag=f"pt{q}",
                                       name=f"pt{q}")
                        nc.scalar.activation(
                            out=pt[:], in_=bp[:], func=Exp, bias=0.0, scale=1.0)
                        pts.append(pt)

                    # ---- mir numerator | denominator via [m|mask] ----
                    # mp rows padded to 256 f32 so each accumulation group
                    # stays inside one 2KB PSUM bank (516B groups at 1548B
                    # offsets silently corrupt across the bank boundary).
                    mp = ps_m.tile([128, 4, 256], F32, tag="mp", name="mp")
                    for q in range(2):
                        for h in range(2):
                            for jc in range(2):
                                nc.tensor.matmul(
                                    mp[:, q * 2 + h, 0:129],
                                    pts[q][:, jc, h * 128:(h + 1) * 128],
                                    mm_p[:, base + q * 2 + jc, :],
                                    start=(jc == 0), stop=(jc == 1))
                    nc.scalar.copy(
                        out=mir_s[:, base:base + 4, :], in_=mp[:, :, 0:129])

                nc.sync.dma_start(
                    out=mir_d[:, blk0:blk0 + 2 * CH, :], in_=mir_s[:])

            nc.sync.dma_start(out=al_d[:], in_=alpha_all[:])
    nc.finalize()
    return nc


def _get_nc():
    if "nc" not in _CACHE:
        _CACHE["nc"] = _build()
    return _CACHE["nc"]


def _to_pblk(a, x):
    """[BL, N, x] -> [128, NBLK, x] partition-major block layout."""
    return np.ascontiguousarray(
        a.reshape(BL, 2, 128, x).transpose(2, 0, 1, 3).reshape(128, NBLK, x))


def _from_pblk(a, x):
    """[128, NBLK, x] -> [BL, N, x]."""
    return a.reshape(128, BL, 2, x).transpose(1, 2, 0, 3).reshape(BL, N, x)


def run(inputs, trace=False, **kw):
    mirror = np.asarray(inputs["mirror_nodes"], dtype=np.float32)
    sat = np.asarray(inputs["satellite_nodes"], dtype=np.float32)
    mask = np.asarray(inputs["satellite_node_mask"])
    Wq1 = np.asarray(inputs["Wq1"], dtype=np.float64)
    Wk1 = np.asarray(inputs["Wk1"], dtype=np.float64)
    Wq2 = np.asarray(inputs["Wq2"], dtype=np.float64)
    Wk2 = np.asarray(inputs["Wk2"], dtype=np.float64)

    scale = 1.0 / math.sqrt(D)
    At = (scale * (Wk1.T @ Wq1)).astype(np.float16)
    Hs = (scale * (Wq2.T @ Wk2)).astype(np.float16)

    m16 = mirror.astype(np.float16)
    s16 = sat.astype(np.float16)
    mbf = mirror.astype(ml_dtypes.bfloat16)
    mm16 = np.concatenate(
        [np.where(mask[..., None], mbf, ml_dtypes.bfloat16(0.0)),
         mask[..., None].astype(ml_dtypes.bfloat16)], axis=2)

    nc = _get_nc()
    in_maps = []
    for c in range(NCORES):
        lo, hi = c * BL, (c + 1) * BL
        in_maps.append({
            "mr": _to_pblk(m16[lo:hi], 128),
            "mm": _to_pblk(mm16[lo:hi], 129),
            "sr": _to_pblk(s16[lo:hi], 128),
            "At": np.ascontiguousarray(At),
            "Hs": np.ascontiguousarray(Hs),
        })
    res = run_bass_kernel_spmd(nc, in_maps, list(range(NCORES)), trace=trace, **kw)

    sat_parts, mir_parts = [], []
    for c, r in enumerate(res.results):
        lo, hi = c * BL, (c + 1) * BL
        # alpha [128, 2*BL] -> [BL, N]
        al = np.asarray(r["alpha"], dtype=np.float32)
        al = al.reshape(128, BL, 2).transpose(1, 2, 0).reshape(BL, N)
        sat_parts.append(sat[lo:hi] + al[..., None] * (mirror[lo:hi] - sat[lo:hi]))
        mir_u = _from_pblk(
            np.asarray(r["mir_out"]).astype(np.float32), 129)
        mir_parts.append(mir_u[..., :128] / mir_u[..., 128:129])
    sat_out = np.concatenate(sat_parts, axis=0)
    mir_out = np.concatenate(mir_parts, axis=0)
    return (sat_out, mir_out), res


def kernel(**inputs):
    out, _ = run(inputs)
    return out
